# revision 3
# baseline (speedup 1.0000x reference)
"""Trainium2 Bass kernel for nn_Encoder_v0_6468220748615.

Math: the reference only returns y, so the NAC chain collapses:
  uni_y    = X @ (Wz2 @ Wz1).T          -> v = (Wz2 @ Wz1)[0]      (32,)
  delta_uni= uni_y @ Wt1.T Wt2.T Wt3.T Wd1.T Wd2.T Wd3.T
           -> u^T = Wd3 Wd2 Wd1 Wt3 Wt2 Wt1                        (512,)
  delta_uni[k] = sum_{t,c} X[k,t,c] * u[t] * v[c]   (rank-1 bilinear form)
  y[k,c] = delta_uni[k]*Wu[c] + baseline[k,c]*(1 + noise[k,c]*1e-3)

The tiny weight chain (a few 512x512 matvecs, ~11 MFLOP) is folded on the
host into wflat = outer(u, v) (16384 floats) and wuv = tanh(wu)*sig(mu).
The device kernel streams all of X (the 134 MB that dominate) and computes
the weighted reduction with a fused DVE tensor_tensor_reduce, sharded
data-parallel over the K axis across 8 NeuronCores.

Per-core layout: X shard (256, 512*32) with k on partitions (contiguous
64KB/row DMA). wflat is replicated to 128 partitions once via a rank-1
PE matmul (ones^T @ wflat) + ACT PSUM->SBUF copies, engines that are
otherwise idle. Main loop: 16 chunks of (128, 2048): DMA -> TTR
(out=scratch, accum_out chained) -> per-128-row epilogue.
"""

import functools

import numpy as np

import concourse.bacc as bacc
import concourse.mybir as mybir
import concourse.tile as tile
from concourse.alu_op_type import AluOpType
from concourse.bass_utils import run_bass_kernel_spmd

NK, NT, NC_, NM = 2048, 512, 32, 512
NOISE_SCALE = 0.001
NCORES = 8
KSH = NK // NCORES          # 256 k-rows per core
P = 128
NBLK = KSH // P             # 2 partition blocks per core
FREE = NT * NC_             # 16384 f32 per k-row
CHUNK = 2048                # free-dim elements per TTR/DMA chunk (1 MiB DMA)
NCH = FREE // CHUNK         # 8 chunks per block
F32 = mybir.dt.float32


def _build():
    nc = bacc.Bacc("TRN2", target_bir_lowering=False, debug=False,
                   num_devices=NCORES)
    xs = nc.dram_tensor("xs", [KSH, FREE], F32, kind="ExternalInput")
    ns = nc.dram_tensor("ns", [KSH, NC_], F32, kind="ExternalInput")
    wflat = nc.dram_tensor("wflat", [1, FREE], F32, kind="ExternalInput")
    wuv = nc.dram_tensor("wuv", [1, NC_], F32, kind="ExternalInput")
    ys = nc.dram_tensor("ys", [KSH, NC_], F32, kind="ExternalOutput")

    act_copy = mybir.ActivationFunctionType.Copy

    with tile.TileContext(nc) as tc:
        with (
            tc.tile_pool(name="const", bufs=1) as cpool,
            tc.tile_pool(name="xin", bufs=4) as xpool,
            tc.tile_pool(name="scratch", bufs=2) as spool,
            tc.tile_pool(name="acc", bufs=2 * NCH) as apool,
            tc.tile_pool(name="small", bufs=2) as mpool,
            tc.tile_pool(name="psum", bufs=1, space="PSUM") as psum,
        ):
            ones = cpool.tile([1, P], F32)
            nc.vector.memset(ones[:], 1.0)

            wf = cpool.tile([1, FREE], F32)
            nc.sync.dma_start(wf[:], wflat[:])
            wu1 = cpool.tile([1, NC_], F32)
            nc.sync.dma_start(wu1[:], wuv[:])

            # Replicate wflat to all 128 partitions: psum = ones^T @ wf slice.
            wrep = cpool.tile([P, FREE], F32)
            for j in range(FREE // 512):
                pt = psum.tile([P, 512], F32, bufs=4)
                nc.tensor.matmul(pt[:], ones[:], wf[:, j * 512:(j + 1) * 512],
                                 start=True, stop=True)
                nc.scalar.copy(wrep[:, j * 512:(j + 1) * 512], pt[:])

            wurep = cpool.tile([P, NC_], F32)
            pu = psum.tile([P, NC_], F32, bufs=1)
            nc.tensor.matmul(pu[:], ones[:], wu1[:], start=True, stop=True)
            nc.scalar.copy(wurep[:], pu[:])

            for b in range(NBLK):
                lastx = None
                # per-chunk partial dots land in columns of acc_all;
                # one tensor_reduce then yields delta_uni for the block.
                # (tensor_tensor_reduce dies on HW; scalar_tensor_tensor
                # with accum_out is the fused multiply+sum that works.)
                acc_all = apool.tile([P, NCH], F32)
                for j in range(NCH):
                    xt = xpool.tile([P, CHUNK], F32)
                    nc.sync.dma_start(
                        xt[:], xs[b * P:(b + 1) * P, j * CHUNK:(j + 1) * CHUNK])
                    sc = spool.tile([P, CHUNK], F32)
                    nc.vector.scalar_tensor_tensor(
                        sc[:], in0=xt[:], scalar=1.0,
                        in1=wrep[:, j * CHUNK:(j + 1) * CHUNK],
                        op0=AluOpType.mult, op1=AluOpType.mult,
                        accum_out=acc_all[:, j:j + 1],
                    )
                    if j == NCH - 1:
                        # lastX = X[:, -1, :] = last 32 cols of the last chunk
                        lastx = mpool.tile([P, NC_], F32)
                        nc.scalar.copy(lastx[:], xt[:, CHUNK - NC_:CHUNK])
                delta = apool.tile([P, 1], F32)
                nc.vector.tensor_reduce(delta[:], acc_all[:],
                                        axis=mybir.AxisListType.X,
                                        op=AluOpType.add)

                nz = mpool.tile([P, NC_], F32)
                nc.sync.dma_start(nz[:], ns[b * P:(b + 1) * P, :])
                nfac = mpool.tile([P, NC_], F32)
                # nfac = 1 + NOISE_SCALE * noise
                nc.scalar.activation(nfac[:], nz[:], act_copy,
                                     bias=1.0, scale=NOISE_SCALE)
                ssum = apool.tile([P, 1], F32)
                nc.vector.tensor_reduce(ssum[:], lastx[:],
                                        axis=mybir.AxisListType.X,
                                        op=AluOpType.add)
                negmean = apool.tile([P, 1], F32)
                nc.scalar.mul(negmean[:], ssum[:], -1.0 / NC_)
                base = mpool.tile([P, NC_], F32)
                nc.vector.tensor_scalar_add(base[:], lastx[:], negmean[:])
                yb = mpool.tile([P, NC_], F32)
                nc.vector.tensor_tensor(yb[:], base[:], nfac[:],
                                        op=AluOpType.mult)
                yt = mpool.tile([P, NC_], F32)
                # yt = wurep * delta_uni + baseline*(1+eps*noise)
                nc.vector.scalar_tensor_tensor(
                    yt[:], in0=wurep[:], scalar=delta[:], in1=yb[:],
                    op0=AluOpType.mult, op1=AluOpType.add)
                nc.sync.dma_start(ys[b * P:(b + 1) * P, :], yt[:])

    nc.compile()
    return nc


@functools.lru_cache(maxsize=1)
def _get_nc():
    return _build()


def _sigmoid(x):
    out = np.empty_like(x)
    pos = x >= 0
    out[pos] = 1.0 / (1.0 + np.exp(-x[pos]))
    ex = np.exp(x[~pos])
    out[~pos] = ex / (1.0 + ex)
    return out


def _nacw(w, m):
    return np.tanh(w) * _sigmoid(m)


def _run(inputs, trace=False, **kwargs):
    X = np.ascontiguousarray(np.asarray(inputs["X"], dtype=np.float32))
    noise = np.ascontiguousarray(np.asarray(inputs["noise"], dtype=np.float32))
    f = {k: np.asarray(inputs[k], dtype=np.float32) for k in (
        "wz1", "mz1", "wz2", "mz2", "wu", "mu",
        "wt1", "mt1", "wt2", "mt2", "wt3", "mt3",
        "wd1", "md1", "wd2", "md2", "wd3", "md3")}

    v = (_nacw(f["wz2"], f["mz2"]) @ _nacw(f["wz1"], f["mz1"]))[0]      # (32,)
    r = _nacw(f["wd3"], f["md3"])                                       # (1,512)
    for wn, mn in (("wd2", "md2"), ("wd1", "md1"), ("wt3", "mt3"),
                   ("wt2", "mt2"), ("wt1", "mt1")):
        r = r @ _nacw(f[wn], f[mn])
    u = r[0]                                                            # (512,)
    wflat = np.ascontiguousarray(
        np.outer(u, v).astype(np.float32).reshape(1, FREE))
    wuv = np.ascontiguousarray(
        _nacw(f["wu"], f["mu"])[:, 0].astype(np.float32).reshape(1, NC_))

    in_maps = []
    for s in range(NCORES):
        in_maps.append({
            "xs": X[s * KSH:(s + 1) * KSH].reshape(KSH, FREE),
            "ns": noise[s * KSH:(s + 1) * KSH],
            "wflat": wflat,
            "wuv": wuv,
        })
    res = run_bass_kernel_spmd(_get_nc(), in_maps,
                               core_ids=list(range(NCORES)),
                               trace=trace, **kwargs)
    y = np.concatenate([r["ys"] for r in res.results], axis=0)
    return y, res


def kernel(**inputs) -> np.ndarray:
    y, _ = _run(inputs)
    return y


# revision 4
# speedup vs baseline: 1.0936x; 1.0936x over previous
"""Trainium2 Bass kernel for nn_Encoder_v0_6468220748615.

Math: the reference only returns y, so the NAC chain collapses:
  uni_y    = X @ (Wz2 @ Wz1).T          -> v = (Wz2 @ Wz1)[0]      (32,)
  delta_uni= uni_y @ Wt1.T Wt2.T Wt3.T Wd1.T Wd2.T Wd3.T
           -> u^T = Wd3 Wd2 Wd1 Wt3 Wt2 Wt1                        (512,)
  delta_uni[k] = sum_{t,c} X[k,t,c] * u[t] * v[c]   (rank-1 bilinear form)
  y[k,c] = delta_uni[k]*Wu[c] + baseline[k,c]*(1 + noise[k,c]*1e-3)

The tiny weight chain (a few 512x512 matvecs, ~11 MFLOP) is folded on the
host into wflat = outer(u, v) (16384 floats) and wuv = tanh(wu)*sig(mu).
The device kernel streams all of X (the 134 MB that dominate) and computes
the weighted reduction with a fused DVE tensor_tensor_reduce, sharded
data-parallel over the K axis across 8 NeuronCores.

Per-core layout: X shard (256, 512*32) with k on partitions (contiguous
64KB/row DMA). wflat is replicated to 128 partitions once via a rank-1
PE matmul (ones^T @ wflat) + ACT PSUM->SBUF copies, engines that are
otherwise idle. Main loop: 16 chunks of (128, 2048): DMA -> TTR
(out=scratch, accum_out chained) -> per-128-row epilogue.
"""

import functools

import numpy as np

import concourse.bacc as bacc
import concourse.mybir as mybir
import concourse.tile as tile
from concourse.alu_op_type import AluOpType
from concourse.bass_utils import run_bass_kernel_spmd

NK, NT, NC_, NM = 2048, 512, 32, 512
NOISE_SCALE = 0.001
NCORES = 8
KSH = NK // NCORES          # 256 k-rows per core
P = 128
NBLK = KSH // P             # 2 partition blocks per core
FREE = NT * NC_             # 16384 f32 per k-row
CHUNK = 2048                # free-dim elements per TTR/DMA chunk (1 MiB DMA)
NCH = FREE // CHUNK         # 8 chunks per block
F32 = mybir.dt.float32


def _build():
    nc = bacc.Bacc("TRN2", target_bir_lowering=False, debug=False,
                   num_devices=NCORES)
    xs = nc.dram_tensor("xs", [KSH, FREE], F32, kind="ExternalInput")
    ns = nc.dram_tensor("ns", [KSH, NC_], F32, kind="ExternalInput")
    wflat = nc.dram_tensor("wflat", [1, FREE], F32, kind="ExternalInput")
    wuv = nc.dram_tensor("wuv", [1, NC_], F32, kind="ExternalInput")
    ys = nc.dram_tensor("ys", [KSH, NC_], F32, kind="ExternalOutput")

    act_copy = mybir.ActivationFunctionType.Copy

    with tile.TileContext(nc) as tc:
        with (
            tc.tile_pool(name="const", bufs=1) as cpool,
            tc.tile_pool(name="xin", bufs=4) as xpool,
            tc.tile_pool(name="scratch", bufs=2) as spool,
            tc.tile_pool(name="acc", bufs=2 * NCH) as apool,
            tc.tile_pool(name="small", bufs=2) as mpool,
        ):
            wf = cpool.tile([1, FREE], F32)
            nc.sync.dma_start(wf[:], wflat[:])
            wu1 = cpool.tile([1, NC_], F32)
            nc.sync.dma_start(wu1[:], wuv[:])

            # Replicate wflat to all 128 partitions on GpSimd, chunked so
            # TTR chunk j only waits on its own slice. (fp32 K=1 PE matmul
            # replication measured ~1us/pass and serialized the kernel.)
            wrep = cpool.tile([P, FREE], F32)
            for j in range(NCH):
                nc.gpsimd.partition_broadcast(
                    wrep[:, j * CHUNK:(j + 1) * CHUNK],
                    wf[:, j * CHUNK:(j + 1) * CHUNK])

            wurep = cpool.tile([P, NC_], F32)
            nc.gpsimd.partition_broadcast(wurep[:], wu1[:])

            for b in range(NBLK):
                lastx = None
                # per-chunk partial dots land in columns of acc_all;
                # one tensor_reduce then yields delta_uni for the block.
                # (tensor_tensor_reduce dies on HW; scalar_tensor_tensor
                # with accum_out is the fused multiply+sum that works.)
                acc_all = apool.tile([P, NCH], F32)
                for j in range(NCH):
                    xt = xpool.tile([P, CHUNK], F32)
                    nc.sync.dma_start(
                        xt[:], xs[b * P:(b + 1) * P, j * CHUNK:(j + 1) * CHUNK])
                    sc = spool.tile([P, CHUNK], F32)
                    nc.vector.scalar_tensor_tensor(
                        sc[:], in0=xt[:], scalar=1.0,
                        in1=wrep[:, j * CHUNK:(j + 1) * CHUNK],
                        op0=AluOpType.mult, op1=AluOpType.mult,
                        accum_out=acc_all[:, j:j + 1],
                    )
                    if j == NCH - 1:
                        # lastX = X[:, -1, :] = last 32 cols of the last chunk
                        lastx = mpool.tile([P, NC_], F32)
                        nc.scalar.copy(lastx[:], xt[:, CHUNK - NC_:CHUNK])
                delta = apool.tile([P, 1], F32)
                nc.vector.tensor_reduce(delta[:], acc_all[:],
                                        axis=mybir.AxisListType.X,
                                        op=AluOpType.add)

                nz = mpool.tile([P, NC_], F32)
                nc.sync.dma_start(nz[:], ns[b * P:(b + 1) * P, :])
                nfac = mpool.tile([P, NC_], F32)
                # nfac = 1 + NOISE_SCALE * noise
                nc.scalar.activation(nfac[:], nz[:], act_copy,
                                     bias=1.0, scale=NOISE_SCALE)
                ssum = apool.tile([P, 1], F32)
                nc.vector.tensor_reduce(ssum[:], lastx[:],
                                        axis=mybir.AxisListType.X,
                                        op=AluOpType.add)
                negmean = apool.tile([P, 1], F32)
                nc.scalar.mul(negmean[:], ssum[:], -1.0 / NC_)
                base = mpool.tile([P, NC_], F32)
                nc.vector.tensor_scalar_add(base[:], lastx[:], negmean[:])
                yb = mpool.tile([P, NC_], F32)
                nc.vector.tensor_tensor(yb[:], base[:], nfac[:],
                                        op=AluOpType.mult)
                yt = mpool.tile([P, NC_], F32)
                # yt = wurep * delta_uni + baseline*(1+eps*noise)
                nc.vector.scalar_tensor_tensor(
                    yt[:], in0=wurep[:], scalar=delta[:], in1=yb[:],
                    op0=AluOpType.mult, op1=AluOpType.add)
                nc.sync.dma_start(ys[b * P:(b + 1) * P, :], yt[:])

    nc.compile()
    return nc


@functools.lru_cache(maxsize=1)
def _get_nc():
    return _build()


def _sigmoid(x):
    out = np.empty_like(x)
    pos = x >= 0
    out[pos] = 1.0 / (1.0 + np.exp(-x[pos]))
    ex = np.exp(x[~pos])
    out[~pos] = ex / (1.0 + ex)
    return out


def _nacw(w, m):
    return np.tanh(w) * _sigmoid(m)


def _run(inputs, trace=False, **kwargs):
    X = np.ascontiguousarray(np.asarray(inputs["X"], dtype=np.float32))
    noise = np.ascontiguousarray(np.asarray(inputs["noise"], dtype=np.float32))
    f = {k: np.asarray(inputs[k], dtype=np.float32) for k in (
        "wz1", "mz1", "wz2", "mz2", "wu", "mu",
        "wt1", "mt1", "wt2", "mt2", "wt3", "mt3",
        "wd1", "md1", "wd2", "md2", "wd3", "md3")}

    v = (_nacw(f["wz2"], f["mz2"]) @ _nacw(f["wz1"], f["mz1"]))[0]      # (32,)
    r = _nacw(f["wd3"], f["md3"])                                       # (1,512)
    for wn, mn in (("wd2", "md2"), ("wd1", "md1"), ("wt3", "mt3"),
                   ("wt2", "mt2"), ("wt1", "mt1")):
        r = r @ _nacw(f[wn], f[mn])
    u = r[0]                                                            # (512,)
    wflat = np.ascontiguousarray(
        np.outer(u, v).astype(np.float32).reshape(1, FREE))
    wuv = np.ascontiguousarray(
        _nacw(f["wu"], f["mu"])[:, 0].astype(np.float32).reshape(1, NC_))

    in_maps = []
    for s in range(NCORES):
        in_maps.append({
            "xs": X[s * KSH:(s + 1) * KSH].reshape(KSH, FREE),
            "ns": noise[s * KSH:(s + 1) * KSH],
            "wflat": wflat,
            "wuv": wuv,
        })
    res = run_bass_kernel_spmd(_get_nc(), in_maps,
                               core_ids=list(range(NCORES)),
                               trace=trace, **kwargs)
    y = np.concatenate([r["ys"] for r in res.results], axis=0)
    return y, res


def kernel(**inputs) -> np.ndarray:
    y, _ = _run(inputs)
    return y
